# revision 23
# baseline (speedup 1.0000x reference)
"""Trainium2 Bass kernel for nn_AttentionBlock (GroupNorm + 4-head attention + proj + residual).

Sharding: data-parallel over batch B=16 across 8 cores (2 batches/core).
Layouts per batch (C=512 -> 4 partition tiles of 128, N=H*W=1024):
  x, h, q, k:  [128, 4(co), 1024]   channel c = co*128 + p
  vT:          [128, 8(nc), 512]    v transposed -> [n, c]; bf16
  P^T (probs): [128, 8(mc), 1024]   exp(scores^T) per head; bf16
  attn out:    [128, 4(head), 1024] normalized attention output; bf16
Scores are computed transposed (S^T[m, n]) so the PV matmul needs no transposes.
Softmax skips max-subtraction (scores bounded ~ +-7.4 for this distribution).
Denominator: DVE reduce over chunk dim + ones-matmul across partitions +
gpsimd partition_broadcast.
"""

import os

import numpy as np
import ml_dtypes

import concourse.bass as bass
import concourse.tile as tile
from concourse import mybir

B = 16
N_CORES = 8
B_LOC = B // N_CORES  # 2
C = 512
HW = 32
N = HW * HW  # 1024
NH = 4  # heads
CH = C // NH  # 128 channels/head
CO = C // 128  # 4 partition tiles over channels
NG = 8  # groups
EPS = 1e-5
SCALE = 1.0 / np.sqrt(CH)

F32 = mybir.dt.float32
BF16 = mybir.dt.bfloat16

_BUILT = None  # cached (nc,)

# Walrus in this toolchain rejects instructions carrying more than a couple of
# embedded sync waits ("Too many sync wait commands"). The Tile end-of-kernel
# drain collects one wait per live proc (11 here). Split them across several
# drain instructions on the sync engine (program order preserves semantics).
_DRAIN_WAIT_LIMIT = 1


def _patch_tile_drain():
    if getattr(tile.TileContext, "_drain_split_patched", False):
        return
    from concourse.vector_clock import ScopedClock

    orig_lower = tile.TileContext._lower_ordered_insts

    def _lower_ordered_insts(self, ordered):
        counter = [0]
        for bbname in list(ordered.keys()):
            insts = ordered[bbname]
            new = []
            for inst in insts:
                si = inst.sync_info
                if (si is not None and si.on_wait and len(si.on_wait) > _DRAIN_WAIT_LIMIT
                        and not str(inst.opcode).startswith("Tile")):
                    waits = list(si.on_wait)
                    chunks = [waits[i:i + _DRAIN_WAIT_LIMIT]
                              for i in range(0, len(waits), _DRAIN_WAIT_LIMIT)]
                    for chunk in chunks[:-1]:
                        nop = mybir.InstNoOp(
                            name=f"waitsplit-{counter[0]}", engine=inst.engine,
                            bass_nofuse=True,
                            sync_info=mybir.SyncInfo(on_wait=chunk, on_update=[]))
                        counter[0] += 1
                        new.append(nop)
                    inst.sync_info = mybir.SyncInfo(
                        on_wait=chunks[-1], on_update=list(si.on_update or []))
                new.append(inst)
            ordered[bbname] = new
        return orig_lower(self, ordered)

    tile.TileContext._lower_ordered_insts = _lower_ordered_insts

    def _drain_and_barrier(self, tick_clock, wait_clock):
        drain_inst = self.nc.sync.drain()
        wait_clock.add_sem_waits(drain_inst.ins, ScopedClock({None: tick_clock.global_clock}))
        si = drain_inst.ins.sync_info
        if si is not None and si.on_wait and len(si.on_wait) > _DRAIN_WAIT_LIMIT:
            waits = list(si.on_wait)
            drain_inst.ins.sync_info = mybir.SyncInfo(
                on_wait=waits[:_DRAIN_WAIT_LIMIT], on_update=list(si.on_update or []))
            for i in range(_DRAIN_WAIT_LIMIT, len(waits), _DRAIN_WAIT_LIMIT):
                extra = self.nc.sync.drain()
                extra.ins.sync_info = mybir.SyncInfo(
                    on_wait=waits[i:i + _DRAIN_WAIT_LIMIT], on_update=[])
        self.nc.all_engine_barrier()
        assert self.sems is not None
        popped = self.nc._tile_sem_poison_stack.pop()
        assert popped is self._sem_poison
        self.nc.clear_and_free_semaphores(list(self.sems.allocated().values()))
        self.nc.all_engine_barrier()

    tile.TileContext._drain_and_barrier = _drain_and_barrier
    tile.TileContext._drain_split_patched = True


def _ns(j):
    """n-half slice."""
    return slice(j * 512, (j + 1) * 512)


def _cs(co):
    """128-wide channel-chunk slice."""
    return slice(co * 128, (co + 1) * 128)


def _emit(tc, aps):
    nc = tc.nc
    import contextlib

    ctx = contextlib.ExitStack()
    with ctx:
        cpool = ctx.enter_context(tc.tile_pool(name="consts", bufs=1))
        xpool = ctx.enter_context(tc.tile_pool(name="x", bufs=2))
        hpool = ctx.enter_context(tc.tile_pool(name="h", bufs=1))
        qpool = ctx.enter_context(tc.tile_pool(name="q", bufs=2))
        kpool = ctx.enter_context(tc.tile_pool(name="k", bufs=2))
        vtpool = ctx.enter_context(tc.tile_pool(name="vt", bufs=2))
        ptpool = ctx.enter_context(tc.tile_pool(name="pt", bufs=2))
        dpool = ctx.enter_context(tc.tile_pool(name="d", bufs=2))
        apool = ctx.enter_context(tc.tile_pool(name="attn", bufs=2))
        opool = ctx.enter_context(tc.tile_pool(name="osb", bufs=1))
        spool = ctx.enter_context(tc.tile_pool(name="stats", bufs=2))
        pmm = ctx.enter_context(tc.tile_pool(name="pmm", bufs=3, space="PSUM"))
        drpool = ctx.enter_context(tc.tile_pool(name="dscratch", bufs=2, space="DRAM"))

        # ---- input x first (it gates the GroupNorm stats critical path)
        x_tiles = []
        for b in range(B_LOC):
            x_t = xpool.tile([128, CO, N], F32, tag="x", name=f"x{b}")
            nc.sync.dma_start(out=x_t, in_=aps["x"][b].rearrange("(co p) n -> p co n", p=128))
            x_tiles.append(x_t)

        # ---- constants into SBUF
        wq_sb = cpool.tile([128, CO, C], BF16, tag="wq")
        wk_sb = cpool.tile([128, CO, C], BF16, tag="wk")
        wv_sb = cpool.tile([128, CO, C], BF16, tag="wv")
        wp_sb = cpool.tile([128, CO, C], BF16, tag="wp")
        for name, t in (("wqt", wq_sb), ("wkt", wk_sb), ("wvt", wv_sb), ("wptb", wp_sb)):
            nc.sync.dma_start(out=t, in_=aps[name].rearrange("(kt p) c -> p kt c", p=128))
        qb_sb = cpool.tile([128, CO], F32, tag="qb")
        kb_sb = cpool.tile([128, CO], F32, tag="kb")
        cb_sb = cpool.tile([128, CO], F32, tag="cb")
        nw_sb = cpool.tile([128, CO], F32, tag="nw")
        nb_sb = cpool.tile([128, CO], F32, tag="nb")
        for name, t in (("qb", qb_sb), ("kb", kb_sb), ("cb", cb_sb), ("nw", nw_sb), ("nbv", nb_sb)):
            nc.sync.dma_start(out=t, in_=aps[name].rearrange("(co p) -> p co", p=128))
        hind_sb = cpool.tile([128, 2], BF16, tag="hind")
        nc.sync.dma_start(out=hind_sb, in_=aps["hind"])
        hindT_sb = cpool.tile([2, 128], BF16, tag="hindT")
        nc.sync.dma_start(out=hindT_sb, in_=aps["hindT"])
        ones_sb = cpool.tile([128, 1], BF16, tag="ones1")
        nc.vector.memset(ones_sb, 1.0)
        eps_sb = cpool.tile([2, 1], F32, tag="eps")
        nc.vector.memset(eps_sb, EPS)

        mult = mybir.AluOpType.mult
        add = mybir.AluOpType.add
        sub = mybir.AluOpType.subtract
        AFT = mybir.ActivationFunctionType

        def emit_stats_and_qkv(b, x_t):
            # ---- GroupNorm stats: per-partition mean/var over N, then combine
            # over the 64-partition half that forms each group.
            mv = spool.tile([128, CO, 2], F32, tag="mv")
            for co in range(CO):
                st = spool.tile([128, 2, 6], F32, tag="bnst")
                xv = x_t[:, co, :].rearrange("p (s f) -> p s f", f=512)
                for sgrp in range(2):
                    nc.vector.bn_stats(out=st[:, sgrp, :], in_=xv[:, sgrp, :])
                nc.vector.bn_aggr(out=mv[:, co, :], in_=st)
            m2 = spool.tile([128, CO], F32, tag="m2")
            nc.vector.tensor_tensor(out=m2, in0=mv[:, :, 0], in1=mv[:, :, 0], op=mult)
            s8 = spool.tile([128, CO, 2], BF16, tag="s8")
            nc.vector.tensor_copy(out=s8[:, :, 0], in_=mv[:, :, 0])
            nc.vector.tensor_tensor(out=s8[:, :, 1], in0=mv[:, :, 1], in1=m2, op=add)
            gs_ps = pmm.tile([2, 2 * CO], F32, tag="mm")
            nc.tensor.matmul(gs_ps, lhsT=hind_sb, rhs=s8.rearrange("p a b -> p (a b)"),
                             start=True, stop=True)
            gmv = spool.tile([2, CO, 2], F32, tag="gmv")
            nc.vector.tensor_scalar_mul(gmv, gs_ps.rearrange("p (a b) -> p a b", b=2), 1.0 / 64.0)
            gm2 = spool.tile([2, CO], F32, tag="gm2")
            nc.vector.tensor_tensor(out=gm2, in0=gmv[:, :, 0], in1=gmv[:, :, 0], op=mult)
            gvar = spool.tile([2, CO], F32, tag="gvar")
            nc.vector.tensor_tensor(out=gvar, in0=gmv[:, :, 1], in1=gm2, op=sub)
            glog = spool.tile([2, CO], F32, tag="glog")
            nc.scalar.activation(glog, gvar, AFT.Ln, bias=eps_sb, scale=1.0)
            grstd = spool.tile([2, CO], F32, tag="grstd")
            nc.scalar.activation(grstd, glog, AFT.Exp, bias=0.0, scale=-0.5)
            gpack = spool.tile([2, CO, 2], BF16, tag="gpack")
            nc.vector.tensor_copy(out=gpack[:, :, 0], in_=gmv[:, :, 0])
            nc.vector.tensor_copy(out=gpack[:, :, 1], in_=grstd)
            bst_ps = pmm.tile([128, 2 * CO], F32, tag="mm")
            nc.tensor.matmul(bst_ps, lhsT=hindT_sb, rhs=gpack.rearrange("p a b -> p (a b)"),
                             start=True, stop=True)
            bs = spool.tile([128, CO, 2], F32, tag="bs")
            nc.vector.tensor_copy(out=bs, in_=bst_ps.rearrange("p (a b) -> p a b", b=2))
            # scale = rstd*w ; shift = mean - b/scale  => h = (x - shift)*scale
            scl = spool.tile([128, CO], F32, tag="scl")
            nc.vector.tensor_tensor(out=scl, in0=bs[:, :, 1], in1=nw_sb, op=mult)
            rscl = spool.tile([128, CO], F32, tag="rscl")
            nc.vector.reciprocal(rscl, scl)
            tmpb = spool.tile([128, CO], F32, tag="tmpb")
            nc.vector.tensor_tensor(out=tmpb, in0=nb_sb, in1=rscl, op=mult)
            shf = spool.tile([128, CO], F32, tag="shf")
            nc.vector.tensor_tensor(out=shf, in0=bs[:, :, 0], in1=tmpb, op=sub)
            h_t = hpool.tile([128, CO, N], BF16, tag="h")
            for co in range(CO):
                nc.vector.tensor_scalar(out=h_t[:, co, :], in0=x_t[:, co, :],
                                        scalar1=shf[:, co:co + 1], scalar2=scl[:, co:co + 1],
                                        op0=sub, op1=mult)

            # ---- q, k projections: q[c, n] accumulated over 4 k-tiles
            q_t = qpool.tile([128, CO, N], BF16, tag="q")
            k_t = kpool.tile([128, CO, N], BF16, tag="k")
            for wsb, bsb, dst in ((wq_sb, qb_sb, q_t), (wk_sb, kb_sb, k_t)):
                for co in range(CO):
                    ps = pmm.tile([128, N], F32, tag="mm")
                    for j in range(2):
                        for kt in range(CO):
                            nc.tensor.matmul(ps[:, _ns(j)], lhsT=wsb[:, kt, _cs(co)],
                                             rhs=h_t[:, kt, _ns(j)],
                                             start=(kt == 0), stop=(kt == CO - 1))
                    nc.scalar.activation(dst[:, co, :], ps, AFT.Identity,
                                         bias=bsb[:, co:co + 1], scale=1.0)

            # ---- vT = h^T @ Wv^T : [n, c] in bf16 (v bias folded into cb on host)
            vt = vtpool.tile([128, 8, C], BF16, tag="vt")
            for mp in range(4):
                ps = pmm.tile([128, N], F32, tag="mm")
                for j in range(2):
                    nchunk = mp * 2 + j
                    for kt in range(CO):
                        nc.tensor.matmul(ps[:, _ns(j)],
                                         lhsT=h_t[:, kt, nchunk * 128:(nchunk + 1) * 128],
                                         rhs=wv_sb[:, kt, :],
                                         start=(kt == 0), stop=(kt == CO - 1))
                nc.scalar.activation(vt[:, mp * 2:(mp + 1) * 2, :],
                                     ps.rearrange("p (a b) -> p a b", a=2), AFT.Copy)
            return q_t, k_t, vt

        def emit_heads(b, q_t, k_t, vt):
            # ---- attention per head (software-pipelined: scores of head hh+1
            # are emitted before the PV of head hh so the PE never stalls on exp)
            attn = apool.tile([128, NH, N], BF16, tag="attn")
            pts = {}
            # Shared denominator psum: head hh's two halves land on partition
            # row 32*hh (tile_position col offsets), so one reciprocal covers
            # the whole batch.
            dallB = pmm.tile([128, N], F32, tag="dallB", bufs=1)
            nc.vector.memset(dallB, 1.0)

            def emit_scores(hh):
                pt = ptpool.tile([128, 8, N], BF16, tag="pt")
                pts[hh] = pt
                for mc in range(8):
                    sps = pmm.tile([128, N], F32, tag="mm")
                    for j in range(2):
                        nc.tensor.matmul(sps[:, _ns(j)],
                                         lhsT=k_t[:, hh, mc * 128:(mc + 1) * 128],
                                         rhs=q_t[:, hh, _ns(j)],
                                         start=True, stop=True)
                    nc.scalar.activation(pt[:, mc, :], sps, AFT.Exp, scale=float(SCALE))

            def emit_pv(hh):
                pt = pts.pop(hh)
                # denominator = sum over all m: bf16 pairwise tree over the 8
                # chunk planes (DVE), then ones-matmul over the 128 partitions
                # into row 32*hh of the shared psum tile.
                ta = [dpool.tile([128, N], BF16, tag=f"dt{i}", name=f"dt{i}", bufs=1)
                      for i in range(2)]
                tb = dpool.tile([128, N], BF16, tag="dt2", bufs=1)
                dsum = dpool.tile([128, N], BF16, tag="dsum")
                nc.vector.tensor_tensor(out=ta[0], in0=pt[:, 0, :], in1=pt[:, 1, :], op=add)
                nc.vector.tensor_tensor(out=ta[1], in0=pt[:, 2, :], in1=pt[:, 3, :], op=add)
                nc.vector.tensor_tensor(out=tb, in0=pt[:, 4, :], in1=pt[:, 5, :], op=add)
                nc.vector.tensor_tensor(out=ta[0], in0=ta[0], in1=ta[1], op=add)
                nc.vector.tensor_tensor(out=ta[1], in0=pt[:, 6, :], in1=pt[:, 7, :], op=add)
                nc.vector.tensor_tensor(out=tb, in0=tb, in1=ta[1], op=add)
                nc.vector.tensor_tensor(out=dsum, in0=ta[0], in1=tb, op=add)
                for j in range(2):
                    nc.tensor.matmul(dallB[32 * hh:32 * hh + 1, _ns(j)], lhsT=ones_sb,
                                     rhs=dsum[:, _ns(j)], start=True, stop=True,
                                     tile_position=(0, 32 * hh))
                # unnormalized PV -> attn (normalized in place once 1/denom is
                # broadcast, after all heads' denominators are in)
                pv = pmm.tile([128, N], F32, tag="mm")
                for j in range(2):
                    for mc in range(8):
                        nc.tensor.matmul(pv[:, _ns(j)], lhsT=vt[:, mc, hh * 128:(hh + 1) * 128],
                                         rhs=pt[:, mc, _ns(j)],
                                         start=(mc == 0), stop=(mc == 7))
                nc.vector.tensor_copy(out=attn[:, hh, :], in_=pv)

            rd128 = dpool.tile([128, N], F32, tag="rd128", bufs=1)
            dn4 = drpool.tile([4, N], F32, tag="dn4")

            def emit_normalize(hh):
                nc.sync.dma_start(out=dn4[hh], in_=rd128[32 * hh:32 * hh + 1, :])
                rdb = dpool.tile([128, N], F32, tag="rdb")
                row = dn4[hh]
                dn_bcast = bass.AP(tensor=row.tensor, offset=row.offset,
                                   ap=[[0, 128]] + list(row.ap))
                nc.sync.dma_start(out=rdb, in_=dn_bcast)
                nc.vector.tensor_tensor(out=attn[:, hh, :], in0=attn[:, hh, :],
                                        in1=rdb, op=mult)

            emit_scores(0)
            emit_scores(1)
            emit_pv(0)
            emit_scores(2)
            emit_pv(1)
            # heads 0/1 denominators are in rows 0/32: invert the lower half
            # early so their normalize overlaps heads 2/3 compute
            nc.vector.reciprocal(rd128[0:64, :], dallB[0:64, :])
            emit_scores(3)
            emit_normalize(0)
            emit_pv(2)
            emit_normalize(1)
            emit_pv(3)
            nc.vector.reciprocal(rd128[64:128, :], dallB[64:128, :])
            emit_normalize(2)
            emit_normalize(3)
            return attn

        def emit_proj(b, x_t, attn):
            # ---- proj + bias (cb = Wp@vb + pb) + residual
            osb = opool.tile([128, CO, N], F32, tag="osb")
            for co in range(CO):
                ps = pmm.tile([128, N], F32, tag="mm")
                for j in range(2):
                    for kt in range(CO):
                        nc.tensor.matmul(ps[:, _ns(j)], lhsT=wp_sb[:, kt, _cs(co)],
                                         rhs=attn[:, kt, _ns(j)],
                                         start=(kt == 0), stop=(kt == CO - 1))
                nc.vector.tensor_scalar(out=osb[:, co, :], in0=ps,
                                        scalar1=cb_sb[:, co:co + 1], scalar2=None,
                                        op0=add)
                nc.vector.tensor_tensor(out=osb[:, co, :], in0=osb[:, co, :],
                                        in1=x_t[:, co, :], op=add)
            nc.sync.dma_start(out=aps["out"][b].rearrange("(co p) n -> p co n", p=128), in_=osb)

        # interleaved schedule: batch 1's stats+qkv fill the PE trough left by
        # batch 0's denominator/normalize tail; proj(b0) overlaps heads(b1) ramp.
        x0, x1 = x_tiles
        qkv0 = emit_stats_and_qkv(0, x0)
        attn0 = emit_heads(0, *qkv0)
        qkv1 = emit_stats_and_qkv(1, x1)
        emit_proj(0, x0, attn0)
        attn1 = emit_heads(1, *qkv1)
        emit_proj(1, x1, attn1)


def build():
    """Build the per-core Bass program (same program on all 8 cores)."""
    _patch_tile_drain()
    nc = bass.Bass("TRN2", target_bir_lowering=False, debug=False)
    aps = {}
    aps["x"] = nc.dram_tensor("x", (B_LOC, C, N), F32, kind="ExternalInput").ap()
    for name in ("wqt", "wkt", "wvt"):
        aps[name] = nc.dram_tensor(name, (C, C), BF16, kind="ExternalInput").ap()
    aps["wptb"] = nc.dram_tensor("wptb", (C, C), BF16, kind="ExternalInput").ap()
    for name in ("qb", "kb", "cb", "nw", "nbv"):
        aps[name] = nc.dram_tensor(name, (C,), F32, kind="ExternalInput").ap()
    aps["hind"] = nc.dram_tensor("hind", (128, 2), BF16, kind="ExternalInput").ap()
    aps["hindT"] = nc.dram_tensor("hindT", (2, 128), BF16, kind="ExternalInput").ap()
    aps["out"] = nc.dram_tensor("out", (B_LOC, C, N), F32, kind="ExternalOutput").ap()
    with tile.TileContext(nc) as tc:
        _emit(tc, aps)
    return nc


def make_in_maps(x, norm_w, norm_b, q_w, q_b, k_w, k_b, v_w, v_b, p_w, p_b):
    """Host-side prep: shard x over 8 cores, pre-transpose weights, fold biases."""
    f = lambda a: np.ascontiguousarray(np.asarray(a, dtype=np.float32))
    x = f(x).reshape(B, C, N)
    wqt = np.ascontiguousarray(f(q_w).T).astype(ml_dtypes.bfloat16)
    wkt = np.ascontiguousarray(f(k_w).T).astype(ml_dtypes.bfloat16)
    wvt = np.ascontiguousarray(f(v_w).T).astype(ml_dtypes.bfloat16)
    wptb = np.ascontiguousarray(f(p_w).T).astype(ml_dtypes.bfloat16)
    cb = f(p_w) @ f(v_b) + f(p_b)
    hind = np.zeros((128, 2), ml_dtypes.bfloat16)
    hind[:64, 0] = 1.0
    hind[64:, 1] = 1.0
    hindT = np.ascontiguousarray(hind.T)
    shared = dict(wqt=wqt, wkt=wkt, wvt=wvt, wptb=wptb, qb=f(q_b), kb=f(k_b),
                  cb=cb, nw=f(norm_w), nbv=f(norm_b), hind=hind, hindT=hindT)
    in_maps = []
    for c in range(N_CORES):
        m = dict(shared)
        m["x"] = np.ascontiguousarray(x[c * B_LOC:(c + 1) * B_LOC])
        in_maps.append(m)
    return in_maps


_last_results = None  # test.py reads this for profile info


def kernel(**inputs) -> np.ndarray:
    global _BUILT, _last_results
    from concourse.bass_utils import run_bass_kernel_spmd

    if _BUILT is None:
        _BUILT = build()
    nc = _BUILT
    in_maps = make_in_maps(**inputs)
    res = run_bass_kernel_spmd(nc, in_maps, core_ids=list(range(N_CORES)))
    _last_results = res
    out = np.concatenate([r["out"] for r in res.results], axis=0)
    return out.reshape(B, C, HW, HW).astype(np.float32)
